# revision 9
# baseline (speedup 1.0000x reference)
"""Causal multi-head attention for Trainium2, SPMD over 8 NeuronCores.

Problem: B=4, H=16, S=2048, Dh=64 fp32.  softmax(Q K^T / sqrt(Dh) + causal) V.

Sharding: the 64 (b, h) head-batches are split 8-per-core (data/head
parallel).  Each core runs an identical single-core kernel on its 8 heads;
no collectives are needed.

Per-core algorithm (all layouts chosen so no operand ever needs a transpose
at matmul time):
  - Q^T, K^T ([Dh, S], head-pairs stacked as [128, S] with head A on
    partitions 0:64 and head B on 64:128) are built once via PE transposes.
  - Logits are computed TRANSPOSED: T[j, i] = sum_d K[j, d] Q[i, d], with
    the two heads of a pair row-packed into the 128x128 PE array
    (contraction is Dh=64, so head A uses array rows 0:64 and head B rows
    64:128 concurrently).
  - exp() runs on ScalarE straight out of PSUM (scale=1/sqrt(Dh) folded in),
    one [128, 1024] instruction covering both heads' tiles.
  - The causal mask only affects diagonal 128x128 blocks; those are zeroed
    post-exp with GpSimd affine_select (exp(-1e9) == 0).
  - PV uses V as the stationary operand augmented with a ones column
    ([128, 65]), so the softmax denominators fall out of the same matmul as
    row 64 of the accumulator: O^T[d, i] and sums[i] together in PSUM.
  - A final PE transpose brings O back to natural [i, d] layout, VectorE
    normalizes by the reciprocal of the sums, and the result DMAs out.

Causality is exploited by only visiting j-tiles with j <= i_max of each
i-chunk (~2x compute saving).  Matmuls run as float32r (reduced-precision
fp32 multiplies at full PE rate for moving dims >= 256).
"""

import os
import sys

for _p in ("/opt/trn_rl_repo", "/opt/pypackages"):
    if os.path.isdir(_p) and _p not in sys.path:
        sys.path.insert(0, _p)

import numpy as np

import concourse.bass as bass
import concourse.tile as tile
from concourse import bacc, mybir
from concourse.masks import make_identity

F32 = mybir.dt.float32
F32R = mybir.dt.float32r

P = 128          # partitions / tile edge
D = 64           # head dim
S_FULL = 2048    # sequence length
HPC = 8          # heads per core
N_CORES = 8
IC = 512         # i-chunk (moving free dim of both matmuls)


def build_nc(n_heads=HPC, seq=S_FULL):
    """Build + compile the per-core Bass program.

    Inputs  q, k, v: [n_heads, seq, 64] fp32.
    Output  out:     [n_heads, seq, 64] fp32.
    """
    assert n_heads % 2 == 0 and seq % IC == 0
    nt = seq // P           # number of 128-wide j-tiles
    ncks = seq // IC        # number of 512-wide i-chunks
    tpc = IC // P           # 128-tiles per i-chunk (4)

    nc = bacc.Bacc("TRN2", target_bir_lowering=False, debug=False)

    q_d = nc.dram_tensor("q", [n_heads, seq, D], F32, kind="ExternalInput").ap()
    k_d = nc.dram_tensor("k", [n_heads, seq, D], F32, kind="ExternalInput").ap()
    v_d = nc.dram_tensor("v", [n_heads, seq, D], F32, kind="ExternalInput").ap()
    o_d = nc.dram_tensor("out", [n_heads, seq, D], F32, kind="ExternalOutput").ap()

    # DRAM views tiled to [128, nt, 64]
    def tview(ap, h):
        return ap[h].rearrange("(t p) d -> p t d", p=P)

    with tile.TileContext(nc) as tc:
        with (
            tc.tile_pool(name="const", bufs=1) as const,
            tc.tile_pool(name="vpool", bufs=1) as vpool,
            tc.tile_pool(name="qknat", bufs=2) as qknat,
            tc.tile_pool(name="qkt", bufs=2) as qkt,
            tc.tile_pool(name="ppool", bufs=3) as ppool,
            tc.tile_pool(name="otpool", bufs=2) as otpool,
            tc.tile_pool(name="osb", bufs=2) as osbp,
            tc.tile_pool(name="qkps", bufs=2, space="PSUM") as qkps,
            tc.tile_pool(name="ops", bufs=2, space="PSUM") as ops,
            tc.tile_pool(name="mps", bufs=2, space="PSUM") as mps,
        ):
            ident = const.tile([P, P], F32)
            make_identity(nc, ident)
            ones = const.tile([P, nt], F32)
            nc.vector.memset(ones[:], 1.0)

            # V for all heads, augmented with a ones column: [128, nt, 65].
            # Stored as float32r (rounded on the cast copy) for the PV matmul.
            vt = []
            for h in range(n_heads):
                vraw = qknat.tile([P, nt, D], F32, tag="nat_a")
                nc.sync.dma_start(vraw[:], tview(v_d, h))
                va = vpool.tile([P, nt, D + 1], F32R, tag=f"v{h}")
                nc.vector.tensor_copy(va[:, :, 0:D], vraw[:])
                nc.vector.tensor_copy(va[:, :, D], ones[:])
                vt.append(va)

            for pair in range(n_heads // 2):
                ha, hb = 2 * pair, 2 * pair + 1

                # ---- load Q, K natural and build stacked Q^T, K^T ----
                # Head A lands on partitions 0:64 (direct DVE copy from the
                # transpose PSUM), head B on 64:128 (via an SBUF->SBUF DMA
                # partition shift — matmul outputs must start at PSUM
                # partition 0, so the transpose cannot target 64:128).
                qT = qkt.tile([P, seq], F32R, tag="qT")
                kT = qkt.tile([P, seq], F32R, tag="kT")
                for (dst, src_d) in ((qT, q_d), (kT, k_d)):
                    nat_a = qknat.tile([P, nt, D], F32, tag="nat_a")
                    nat_b = qknat.tile([P, nt, D], F32, tag="nat_b")
                    nc.sync.dma_start(nat_a[:], tview(src_d, ha))
                    nc.sync.dma_start(nat_b[:], tview(src_d, hb))
                    for g in range(nt // 4):
                        tp = qkps.tile([P, 1024], F32, tag="qk")
                        for boff, nat in ((0, nat_a), (512, nat_b)):
                            for u in range(4):
                                jt = 4 * g + u
                                nc.tensor.transpose(
                                    tp[0:D, boff + 128 * u:boff + 128 * (u + 1)],
                                    nat[:, jt, :],
                                    ident,
                                )
                        nc.vector.tensor_copy(
                            dst[0:D, 512 * g:512 * (g + 1)], tp[0:D, 0:512])
                        stage = otpool.tile([P, 512], F32R, tag="stage")
                        nc.vector.tensor_copy(stage[0:D, :], tp[0:D, 512:1024])
                        nc.sync.dma_start(
                            dst[D:P, 512 * g:512 * (g + 1)], stage[0:D, :])

                # ---- attention over i-chunks ----
                for c in range(ncks):
                    oa = ops.tile([P, IC], F32, tag="o")
                    ob = ops.tile([P, IC], F32, tag="o")
                    njt = min(nt, tpc * (c + 1))
                    for jt in range(njt):
                        qk = qkps.tile([P, 2 * IC], F32, tag="qk")
                        js = slice(P * jt, P * (jt + 1))
                        cs = slice(IC * c, IC * (c + 1))
                        nc.tensor.matmul(
                            qk[:, 0:IC], kT[0:D, js], qT[0:D, cs],
                            start=True, stop=True, tile_position=(0, 0),
                        )
                        nc.tensor.matmul(
                            qk[:, IC:2 * IC], kT[D:P, js], qT[D:P, cs],
                            start=True, stop=True, tile_position=(64, 0),
                        )
                        pT = ppool.tile([P, 2 * IC], F32R, tag="pT")
                        nc.scalar.activation(
                            pT[:], qk[:],
                            mybir.ActivationFunctionType.Exp,
                            scale=1.0 / np.sqrt(D),
                        )
                        if P * jt >= IC * c:  # diagonal tile: zero where j > i
                            off = P * jt - IC * c
                            for hoff in (0, IC):
                                # keep (j <= i): j = 128*jt + p, i = 512*c + y
                                # iota = -p + y - off >= 0
                                sl = pT[:, hoff:hoff + off + P]
                                nc.gpsimd.affine_select(
                                    out=sl, in_=sl,
                                    compare_op=mybir.AluOpType.is_ge,
                                    fill=0.0, base=-off,
                                    pattern=[[1, off + P]], channel_multiplier=-1,
                                )
                        st, sp = jt == 0, jt == njt - 1
                        nc.tensor.matmul(
                            oa[0:D + 1, :], vt[ha][:, jt, :], pT[:, 0:IC],
                            start=st, stop=sp,
                        )
                        nc.tensor.matmul(
                            ob[0:D + 1, :], vt[hb][:, jt, :], pT[:, IC:2 * IC],
                            start=st, stop=sp,
                        )

                    # ---- finalize chunk: transpose back, normalize, store ----
                    for h, o_ps in ((ha, oa), (hb, ob)):
                        ot = otpool.tile([P, IC], F32, tag="ot")
                        nc.vector.tensor_copy(ot[0:D + 1, :], o_ps[0:D + 1, :])
                        fin = mps.tile([P, 512], F32, tag="mps")
                        finv = fin[:, 0:tpc * (D + 1)].rearrange(
                            "p (t e) -> p t e", e=D + 1)
                        for t in range(tpc):
                            nc.tensor.transpose(
                                finv[:, t, :],
                                ot[0:D + 1, P * t:P * (t + 1)],
                                ident[0:D + 1, 0:D + 1],
                            )
                        rec = osbp.tile([P, tpc], F32, tag="rec")
                        nc.vector.reciprocal(rec[:], finv[:, :, D])
                        o_sb = osbp.tile([P, tpc, D], F32, tag="osb")
                        nc.vector.tensor_tensor(
                            o_sb[:], finv[:, :, 0:D],
                            rec[:, :, None].to_broadcast([P, tpc, D]),
                            mybir.AluOpType.mult,
                        )
                        nc.sync.dma_start(
                            tview(o_d, h)[:, tpc * c:tpc * (c + 1), :], o_sb[:])

    nc.compile()
    return nc


_NC_CACHE = {}


def _get_nc(n_heads, seq):
    key = (n_heads, seq)
    if key not in _NC_CACHE:
        _NC_CACHE[key] = build_nc(n_heads, seq)
    return _NC_CACHE[key]


def kernel(q, k, v, mask=None, _trace=False):
    """Full-input entry point: q,k,v [4,16,2048,64] fp32 (+ mask, unused:
    causality is applied on-device).  Returns [4,16,2048,64] fp32."""
    from concourse.bass_utils import run_bass_kernel_spmd

    B, H, S, Dh = q.shape
    G = B * H
    gpc = G // N_CORES
    qf = np.ascontiguousarray(q.reshape(G, S, Dh), dtype=np.float32)
    kf = np.ascontiguousarray(k.reshape(G, S, Dh), dtype=np.float32)
    vf = np.ascontiguousarray(v.reshape(G, S, Dh), dtype=np.float32)

    nc = _get_nc(gpc, S)
    in_maps = [
        {
            "q": qf[i * gpc:(i + 1) * gpc],
            "k": kf[i * gpc:(i + 1) * gpc],
            "v": vf[i * gpc:(i + 1) * gpc],
        }
        for i in range(N_CORES)
    ]
    res = run_bass_kernel_spmd(
        nc, in_maps, core_ids=list(range(N_CORES)), trace=_trace)
    out = np.concatenate([res.results[i]["out"] for i in range(N_CORES)], axis=0)
    kernel._last_exec_time_ns = res.exec_time_ns
    return out.reshape(B, H, S, Dh)


# revision 20
# speedup vs baseline: 1.2334x; 1.2334x over previous
"""Causal multi-head attention for Trainium2, SPMD over 8 NeuronCores.

Problem: B=4, H=16, S=2048, Dh=64 fp32.  softmax(Q K^T / sqrt(Dh) + causal) V.

Sharding: the 64 (b, h) head-batches are split 8-per-core (data/head
parallel).  Each core runs an identical single-core kernel on its 8 heads;
no collectives are needed.

Per-core algorithm (all layouts chosen so no operand ever needs a transpose
at matmul time):
  - Q^T, K^T ([Dh, S], head-pairs stacked as [128, S] with head A on
    partitions 0:64 and head B on 64:128) are built once via PE transposes.
  - Logits are computed TRANSPOSED: T[j, i] = sum_d K[j, d] Q[i, d], with
    the two heads of a pair row-packed into the 128x128 PE array
    (contraction is Dh=64, so head A uses array rows 0:64 and head B rows
    64:128 concurrently).
  - exp() runs on ScalarE straight out of PSUM (scale=1/sqrt(Dh) folded in),
    one [128, 1024] instruction covering both heads' tiles.
  - The causal mask only affects diagonal 128x128 blocks; those are zeroed
    post-exp with GpSimd affine_select (exp(-1e9) == 0).
  - PV uses V as the stationary operand augmented with a ones column
    ([128, 65]), so the softmax denominators fall out of the same matmul as
    row 64 of the accumulator: O^T[d, i] and sums[i] together in PSUM.
  - A final PE transpose brings O back to natural [i, d] layout, VectorE
    normalizes by the reciprocal of the sums, and the result DMAs out.

Causality is exploited by only visiting j-tiles with j <= i_max of each
i-chunk (~2x compute saving).  Matmuls run as float32r (reduced-precision
fp32 multiplies at full PE rate for moving dims >= 256).
"""

import os
import sys

for _p in ("/opt/trn_rl_repo", "/opt/pypackages"):
    if os.path.isdir(_p) and _p not in sys.path:
        sys.path.insert(0, _p)

import numpy as np

import concourse.bass as bass
import concourse.tile as tile
from concourse import bacc, mybir
from concourse.masks import make_identity

F32 = mybir.dt.float32
F32R = mybir.dt.float32r

P = 128          # partitions / tile edge
D = 64           # head dim
S_FULL = 2048    # sequence length
HPC = 8          # heads per core
N_CORES = 8
IC = 512         # i-chunk (moving free dim of both matmuls)


def build_nc(n_heads=HPC, seq=S_FULL, skip=()):
    """Build + compile the per-core Bass program.

    Inputs  q, k, v: [n_heads, seq, 64] fp32.
    Output  out:     [n_heads, seq, 64] fp32.
    skip: ablation switches for cost attribution —
          subsets of {"exp", "mask", "pv", "qk", "pro", "fin"}.
    """
    assert n_heads % 2 == 0 and seq % IC == 0
    nt = seq // P           # number of 128-wide j-tiles
    ncks = seq // IC        # number of 512-wide i-chunks
    tpc = IC // P           # 128-tiles per i-chunk (4)

    nc = bacc.Bacc("TRN2", target_bir_lowering=False, debug=False)

    q_d = nc.dram_tensor("q", [n_heads, seq, D], F32, kind="ExternalInput").ap()
    k_d = nc.dram_tensor("k", [n_heads, seq, D], F32, kind="ExternalInput").ap()
    v_d = nc.dram_tensor("v", [n_heads, seq, D], F32, kind="ExternalInput").ap()
    o_d = nc.dram_tensor("out", [n_heads, seq, D], F32, kind="ExternalOutput").ap()

    # DRAM views tiled to [128, nt, 64]
    def tview(ap, h):
        return ap[h].rearrange("(t p) d -> p t d", p=P)

    with tile.TileContext(nc) as tc:
        with (
            tc.tile_pool(name="const", bufs=1) as const,
            tc.tile_pool(name="vpool", bufs=1) as vpool,
            tc.tile_pool(name="qknat", bufs=2) as qknat,
            tc.tile_pool(name="qkt", bufs=2) as qkt,
            tc.tile_pool(name="ppool", bufs=3) as ppool,
            tc.tile_pool(name="otpool", bufs=2) as otpool,
            tc.tile_pool(name="osb", bufs=2) as osbp,
            tc.tile_pool(name="qkps", bufs=2, space="PSUM") as qkps,
            tc.tile_pool(name="ops", bufs=3, space="PSUM") as ops,
            tc.tile_pool(name="pps", bufs=1, space="PSUM") as pps,
        ):
            ident = const.tile([P, P], F32)
            make_identity(nc, ident)
            ones = const.tile([P, nt], F32)
            nc.vector.memset(ones[:], 1.0)

            # V for all heads, augmented with a ones column: [128, nt, 65].
            # Stored as float32r (rounded on the cast copy) for the PV matmul.
            vt = {}

            for pair in range(n_heads // 2):
                ha, hb = 2 * pair, 2 * pair + 1

                # ---- load Q, K natural and build stacked Q^T, K^T ----
                # Head A lands on partitions 0:64 (direct DVE copy from the
                # transpose PSUM), head B on 64:128 (via an SBUF->SBUF DMA
                # partition shift — matmul outputs must start at PSUM
                # partition 0, so the transpose cannot target 64:128).
                qT = qkt.tile([P, seq], F32R, tag="qT")
                kT = qkt.tile([P, seq], F32R, tag="kT")
                for (dst, src_d) in () if "pro" in skip else ((qT, q_d), (kT, k_d)):
                    nat_a = qknat.tile([P, nt, D], F32, tag="nat_a")
                    nat_b = qknat.tile([P, nt, D], F32, tag="nat_b")
                    nc.sync.dma_start(nat_a[:], tview(src_d, ha))
                    nc.sync.dma_start(nat_b[:], tview(src_d, hb))
                    stage = otpool.tile([P, seq], F32R, tag="stage")
                    for g in range(nt // 4):
                        for hh, nat in ((0, nat_a), (1, nat_b)):
                            tp = pps.tile([P, 512], F32, tag="pps")
                            for u in range(4):
                                jt = 4 * g + u
                                nc.tensor.transpose(
                                    tp[0:D, 128 * u:128 * (u + 1)],
                                    nat[:, jt, :],
                                    ident,
                                )
                            if hh == 0:
                                nc.vector.tensor_copy(
                                    dst[0:D, 512 * g:512 * (g + 1)], tp[0:D, :])
                            else:
                                nc.vector.tensor_copy(
                                    stage[0:D, 512 * g:512 * (g + 1)], tp[0:D, :])
                                nc.sync.dma_start(
                                    dst[D:P, 512 * g:512 * (g + 1)],
                                    stage[0:D, 512 * g:512 * (g + 1)])
                for h in (ha, hb):
                    if h not in vt:
                        vraw = qknat.tile([P, nt, D], F32, tag="vraw")
                        nc.sync.dma_start(vraw[:], tview(v_d, h))
                        va = vpool.tile([P, nt, D + 1], F32R, tag=f"v{h}")
                        nc.vector.tensor_copy(va[:, :, 0:D], vraw[:])
                        nc.vector.tensor_copy(va[:, :, D], ones[:])
                        vt[h] = va

                # ---- attention over i-chunks ----
                oacc_a = osbp.tile([P, nt, D], F32, tag="oacc")
                oacc_b = osbp.tile([P, nt, D], F32, tag="oacc")
                oacc = {ha: oacc_a, hb: oacc_b}
                pending_fin = []
                for c in range(ncks):
                    oa = ops.tile([P, IC], F32, tag="o")
                    ob = ops.tile([P, IC], F32, tag="o")
                    njt = min(nt, tpc * (c + 1))
                    for jt in range(njt):
                        # within chunk c, tile jt only matters for
                        # i_local >= off (causality); everything below is
                        # restricted to that live range.
                        off = max(0, P * jt - IC * c)
                        qk = qkps.tile([P, 2 * IC], F32, tag="qk")
                        js = slice(P * jt, P * (jt + 1))
                        cs = slice(IC * c + off, IC * (c + 1))
                        if "qk" not in skip:
                            nc.tensor.matmul(
                                qk[:, off:IC], kT[0:D, js], qT[0:D, cs],
                                start=True, stop=True, tile_position=(0, 0),
                            )
                            nc.tensor.matmul(
                                qk[:, IC + off:2 * IC], kT[D:P, js], qT[D:P, cs],
                                start=True, stop=True, tile_position=(64, 0),
                            )
                        pT = ppool.tile([P, 2 * IC], F32R, tag="pT")
                        if "exp" not in skip:
                            if off == 0:
                                nc.scalar.activation(
                                    pT[:], qk[:],
                                    mybir.ActivationFunctionType.Exp,
                                    scale=1.0 / np.sqrt(D),
                                )
                            else:
                                for hoff in (0, IC):
                                    nc.scalar.activation(
                                        pT[:, hoff + off:hoff + IC],
                                        qk[:, hoff + off:hoff + IC],
                                        mybir.ActivationFunctionType.Exp,
                                        scale=1.0 / np.sqrt(D),
                                    )
                        if P * jt >= IC * c and "mask" not in skip:
                            # diagonal 128x128 block: zero where j > i
                            # (i = 512c + off + y, j = 128jt + p -> keep p <= y)
                            for hoff in (0, IC):
                                sl = pT[:, hoff + off:hoff + off + P]
                                nc.gpsimd.affine_select(
                                    out=sl, in_=sl,
                                    compare_op=mybir.AluOpType.is_ge,
                                    fill=0.0, base=0,
                                    pattern=[[1, P]], channel_multiplier=-1,
                                )
                        st, sp = jt == 0, jt == njt - 1
                        if "pv" not in skip:
                            nc.tensor.matmul(
                                oa[0:D + 1, off:], vt[ha][:, jt, :],
                                pT[:, off:IC], start=st, stop=sp,
                            )
                            nc.tensor.matmul(
                                ob[0:D + 1, off:], vt[hb][:, jt, :],
                                pT[:, IC + off:2 * IC], start=st, stop=sp,
                            )
                        if jt == 1 and pending_fin:
                            pending_fin.pop(0)()

                    # ---- finalize chunk: copy the accumulators out now (so
                    # their PSUM slots free up), but DEFER emission of the
                    # transpose/normalize/store tail until after the next
                    # chunk's first matmuls — the PE then prefers fresh QK/PV
                    # work over finalize transposes at the chunk boundary.
                    ots = {}
                    for h, o_ps in () if "fin" in skip else ((ha, oa), (hb, ob)):
                        ot = otpool.tile([P, IC], F32, tag="ot")
                        nc.vector.tensor_copy(ot[0:D + 1, :], o_ps[0:D + 1, :])
                        ots[h] = ot

                    def _fin(c=c, ots=ots):
                        for h, ot in ots.items():
                            fin = ops.tile([P, IC], F32, tag="o", name=f"fin{c}{h}")
                            finv = fin[:, 0:tpc * (D + 1)].rearrange(
                                "p (t e) -> p t e", e=D + 1)
                            for t in range(tpc):
                                nc.tensor.transpose(
                                    finv[:, t, :],
                                    ot[0:D + 1, P * t:P * (t + 1)],
                                    ident[0:D + 1, 0:D + 1],
                                )
                            rec = osbp.tile([P, tpc], F32, tag="rec",
                                            name=f"rec{c}{h}")
                            nc.vector.reciprocal(rec[:], finv[:, :, D])
                            nc.vector.tensor_tensor(
                                oacc[h][:, tpc * c:tpc * (c + 1), :],
                                finv[:, :, 0:D],
                                rec[:, :, None].to_broadcast([P, tpc, D]),
                                mybir.AluOpType.mult,
                            )
                            nc.scalar.dma_start(
                                tview(o_d, h)[:, tpc * c:tpc * (c + 1), :],
                                oacc[h][:, tpc * c:tpc * (c + 1), :])
                    if "fin" not in skip:
                        if "defer" in skip:
                            pending_fin.append(_fin)
                        else:
                            _fin()
                for f in pending_fin:
                    f()

    nc.compile()
    return nc


_NC_CACHE = {}


def _get_nc(n_heads, seq):
    key = (n_heads, seq)
    if key not in _NC_CACHE:
        _NC_CACHE[key] = build_nc(n_heads, seq)
    return _NC_CACHE[key]


def kernel(q, k, v, mask=None, _trace=False):
    """Full-input entry point: q,k,v [4,16,2048,64] fp32 (+ mask, unused:
    causality is applied on-device).  Returns [4,16,2048,64] fp32."""
    from concourse.bass_utils import run_bass_kernel_spmd

    B, H, S, Dh = q.shape
    G = B * H
    gpc = G // N_CORES
    qf = np.ascontiguousarray(q.reshape(G, S, Dh), dtype=np.float32)
    kf = np.ascontiguousarray(k.reshape(G, S, Dh), dtype=np.float32)
    vf = np.ascontiguousarray(v.reshape(G, S, Dh), dtype=np.float32)

    nc = _get_nc(gpc, S)
    in_maps = [
        {
            "q": qf[i * gpc:(i + 1) * gpc],
            "k": kf[i * gpc:(i + 1) * gpc],
            "v": vf[i * gpc:(i + 1) * gpc],
        }
        for i in range(N_CORES)
    ]
    res = run_bass_kernel_spmd(
        nc, in_maps, core_ids=list(range(N_CORES)), trace=_trace)
    out = np.concatenate([res.results[i]["out"] for i in range(N_CORES)], axis=0)
    kernel._last_exec_time_ns = res.exec_time_ns
    return out.reshape(B, H, S, Dh)
